# revision 38
# baseline (speedup 1.0000x reference)
"""Trainium2 Bass kernel for nn_MultiHeadAttention_67757404062370.

Sharding: data-parallel over batch (2) x tensor-parallel over heads (4 groups
of 4 heads) = 8 NeuronCores. Core c handles batch c//4, heads 4*(c%4)..4*(c%4)+3.

Device-side per core (transposed layout throughout):
  kk^T = Wk_g^T k^T (+bk)  [128=2 heads x 64, 2, seq] fp32r
  qq^T = Wq_g^T q^T (+bq)  [128, 4, seq] fp32r, zero-padded in the other
         head's 64 rows so the scores contraction can use K=128 (K=64
         matmuls never warm the PE clock gate).
  vv   = v Wv_g  [seq, 4*65] fp16 with ones columns
  s^T[k,q] = kkT_tile.T @ qqT_pad   (K=128, fp32r)
  u = exp(s/8) fp16; t = max(u,1); e = t*keep  (exp(relu(x)) == max(exp(x),1))
  av^T (+ sums row via ones cols) = vv_aug.T @ e   (fp16, K=128)
  att^T = e * (1/sums) fp16 ; oav^T = av^T * (1/sums) fp32r
  outp[q,:] = sum_h oav_h^T.T @ Wo_rows_h  (K=64 per head, fp32r)
Host: gather, transpose att views, sum outp over the 4 head-group cores per
batch, add (bv @ Wo + bo), cast to float64.
"""

import sys

if "/opt/trn_rl_repo" not in sys.path:
    sys.path.insert(0, "/opt/trn_rl_repo")

import numpy as np

import concourse.bacc as bacc
import concourse.tile as tile
from concourse import mybir
from concourse.bass_utils import run_bass_kernel_spmd

F = mybir.dt.float32
FR = mybir.dt.float32r
F16 = mybir.dt.float16
AF = mybir.ActivationFunctionType
OP = mybir.AluOpType

S = 1024
DIN = 1024
NH = 4
DEP = 64
DCOL = NH * DEP
NT = S // 128
KT = DIN // 128

_CACHE = {}
_DEBUG = False
_IDENT = np.eye(128, dtype=np.float16)


def _build():
    nc = bacc.Bacc("TRN2", target_bir_lowering=False, debug=False, num_devices=8)

    d = {}
    # packed per-ktile input streams: one large DMA per k-tile
    d["qkw"] = nc.dram_tensor("qkw", [KT, 128, 2560], F16, kind="ExternalInput").ap()
    d["kvw"] = nc.dram_tensor("kvw", [KT, 128, 2304], F16, kind="ExternalInput").ap()
    d["wo"] = nc.dram_tensor("wo", [128, NH, DIN], FR, kind="ExternalInput").ap()
    d["bq"] = nc.dram_tensor("bq", [DEP, NH], F, kind="ExternalInput").ap()
    d["bk"] = nc.dram_tensor("bk", [128, 2], F, kind="ExternalInput").ap()
    d["ident"] = nc.dram_tensor("ident", [128, 128], F16, kind="ExternalInput").ap()
    d["attT"] = nc.dram_tensor("attT", [NH, S, S], F16, kind="ExternalOutput").ap()
    d["outp"] = nc.dram_tensor("outp", [S, DIN], F, kind="ExternalOutput").ap()
    if _DEBUG:
        d["dbg_av"] = nc.dram_tensor("dbg_av", [NH, 65, S], F, kind="ExternalOutput").ap()
        d["dbg_oav"] = nc.dram_tensor("dbg_oav", [NH, DEP, S], F, kind="ExternalOutput").ap()

    with tile.TileContext(nc) as tc:
        _emit(nc, tc, d)
    nc.compile()
    return nc


def _emit(nc, tc, d):
    from contextlib import ExitStack

    ctx = ExitStack()
    with ctx:
        # ---------------- persistent tiles ----------------
        persist = ctx.enter_context(tc.tile_pool(name="persist", bufs=1))
        # q proj, zero-padded per head to a full 128-row contraction
        qqT = persist.tile([128, NH, S], FR, tag="qqT")
        # k proj, two heads stacked per dcol-tile
        kkT = persist.tile([128, 2, S], FR, tag="kkT")
        vv = persist.tile([128, NT, NH * 65], F16, tag="vv")
        wo_s = persist.tile([128, NH, DIN], FR, tag="wo")
        kvw_s = persist.tile([128, KT, 2304], F16, tag="kvw")
        keep_s = [kvw_s[:, i, 0:S] for i in range(KT)]
        vt_s = [kvw_s[:, i, S:2 * S] for i in range(KT)]
        wv_sl = [kvw_s[:, i, 2 * S:2 * S + DCOL] for i in range(KT)]
        mneg_s = [persist.tile([128, S], F16, tag=f"mneg{i}", name=f"mneg{i}") for i in range(KT)]
        ident_s = persist.tile([128, 128], F16, tag="ident")
        bq_s = persist.tile([DEP, NH], F, tag="bq")
        bk_s = persist.tile([128, 2], F, tag="bk")
        ones_sb = persist.tile([128, NH, 1], F, tag="ones")
        nc.vector.memset(ones_sb, 1.0)
        zscr = persist.tile([128, S], F, tag="zscr")
        nc.vector.memset(zscr, 0.0)
        m60k = persist.tile([128, 1], F, tag="m60k")
        nc.vector.memset(m60k, -60000.0)
        ones_row = persist.tile([1, 128], F, tag="ones_row")
        nc.vector.memset(ones_row, 1.0)
        ones_fr = persist.tile([1, 128], FR, tag="ones_fr")
        nc.vector.tensor_copy(out=ones_fr, in_=ones_row)
        oav = [persist.tile([128, S], FR, tag=f"oav{h}", name=f"oav{h}") for h in range(NH)]
        osb_t = [persist.tile([128, DIN], F, tag=f"osb{i}", name=f"osb{i}") for i in range(3)]

        nc.sync.dma_start(out=bq_s, in_=d["bq"])
        nc.sync.dma_start(out=ident_s, in_=d["ident"])
        # zero fills from the scratch, before any data arrives
        for h in range(NH):
            zw = slice((1 - (h % 2)) * DEP, (2 - (h % 2)) * DEP)
            nc.vector.tensor_copy(out=qqT[zw, h, :], in_=zscr[zw, :])
            nc.vector.tensor_copy(out=oav[h][DEP:128, :], in_=zscr[DEP:128, :])
        nc.sync.dma_start(out=bk_s, in_=d["bk"])

        # ---------------- phase A: q/k projections ----------------
        with tc.tile_pool(name="projin", bufs=4) as pin:
            with tc.tile_pool(name="psA1", bufs=4, space="PSUM") as psA1:
                ps_q = [psA1.tile([128, S], F, tag="psA1", name=f"psq{i}") for i in range(2)]
                ps_k = [psA1.tile([128, S], F, tag="psA1", name=f"psk{i}") for i in range(2)]
                for kt in range(KT):
                    qkw = pin.tile([128, 2560], F16, tag="pin", name=f"qkw{kt}")
                    nc.sync.dma_start(out=qkw, in_=d["qkw"][kt])
                    for dc in range(2):
                        for cc in range(2):
                            sl = slice(512 + cc * 512, 512 + (cc + 1) * 512)
                            nc.tensor.matmul(
                                ps_q[dc][:, slice(cc * 512, (cc + 1) * 512)],
                                qkw[:, dc * 128:(dc + 1) * 128],
                                qkw[:, sl],
                                start=(kt == 0), stop=(kt == KT - 1))
                            nc.tensor.matmul(
                                ps_k[dc][:, slice(cc * 512, (cc + 1) * 512)],
                                qkw[:, 256 + dc * 128:256 + (dc + 1) * 128],
                                qkw[:, 1024 + sl.start:1024 + sl.stop],
                                start=(kt == 0), stop=(kt == KT - 1))
                # mask/v/wv loads queue behind the q/k stream
                for kt in range(KT):
                    nc.sync.dma_start(out=kvw_s[:, kt, :], in_=d["kvw"][kt])
                # evacuate: kk on DVE, qq on ACT — both unblock scores fast
                for dc in range(2):
                    nc.vector.tensor_scalar_add(
                        out=kkT[:, dc, :], in0=ps_k[dc], scalar1=bk_s[:, dc:dc + 1])
                    for hf in range(2):
                        h = dc * 2 + hf
                        rw = slice(hf * DEP, (hf + 1) * DEP)
                        nc.scalar.activation(
                            out=qqT[rw, h, :], in_=ps_q[dc][rw, :],
                            func=AF.Identity, bias=bq_s[:, h:h + 1], scale=1.0)


        # wo: host-packed with duplicated head rows (see phase C)
        if True:
            nc.sync.dma_start(out=wo_s, in_=d["wo"])

            # ---------------- phase B: heads in pairs ----------------
            with tc.tile_pool(name="escore", bufs=16) as epool, \
                 tc.tile_pool(name="utile", bufs=3) as upool, \
                 tc.tile_pool(name="attsb", bufs=5) as apool, \
                 tc.tile_pool(name="rtiles", bufs=2) as rpool, \
                 tc.tile_pool(name="psS", bufs=2, space="PSUM") as psS, \
                 tc.tile_pool(name="psAV", bufs=2, space="PSUM") as psAV:
                pending = []  # deferred DVE tail work from the previous pair
                e_tiles_all = {}
                for hp in range(NH // 2):
                    heads = (2 * hp, 2 * hp + 1)
                    e_tiles = {h: [] for h in heads}
                    # scores + exp + mask for both heads of the pair
                    for kt in range(KT):
                        if hp == 0:
                            # kbig = keep * 60000 (mask gate for the min step)
                            nc.vector.tensor_scalar_mul(
                                out=mneg_s[kt], in0=keep_s[kt], scalar1=60000.0)
                        for h in heads:
                            if pending:
                                pending.pop(0)()
                            ps = psS.tile([128, S], F, tag="score", name=f"sc{h}_{kt}")
                            for cc in range(2):
                                sl = slice(cc * 512, (cc + 1) * 512)
                                nc.tensor.matmul(
                                    ps[:, sl],
                                    kkT[:, hp, kt * 128:(kt + 1) * 128],
                                    qqT[:, h, sl],
                                    start=True, stop=True)
                            u = upool.tile([128, S], F16, tag="u", name=f"u{h}_{kt}")
                            nc.scalar.activation(out=u, in_=ps, func=AF.Exp, scale=0.125)
                            # exp-domain relu: t = max(u, 1); mask: e = min(t, keep*60000)
                            t = upool.tile([128, S], F16, tag="t", name=f"t{h}_{kt}")
                            nc.vector.tensor_scalar_max(out=t, in0=u, scalar1=1.0)
                            e = epool.tile([128, S], F16, tag="e", name=f"e{h}_{kt}")
                            nc.vector.tensor_tensor(
                                out=e, in0=t, in1=mneg_s[kt], op=OP.min)
                            e_tiles[h].append(e)
                            e_tiles_all[h] = e_tiles[h]
                        # vv projection interleaved into the back half of S0:
                        # fills the PE's exp-wait gaps; av-pool slots rotate
                        if hp == 0 and kt >= KT // 2:
                            for st in (2 * (kt - KT // 2), 2 * (kt - KT // 2) + 1):
                                pv = psAV.tile([128, 256], F, tag="av", name=f"psv{st}")
                                for vkt in range(KT):
                                    nc.tensor.matmul(
                                        pv,
                                        vt_s[vkt][:, st * 128:(st + 1) * 128],
                                        wv_sl[vkt],
                                        start=(vkt == 0), stop=(vkt == KT - 1))
                                dst = vv[:, st, :].rearrange("p (h x) -> p h x", h=NH)
                                nc.scalar.activation(
                                    out=dst[:, :, 0:DEP],
                                    in_=pv.rearrange("p (h x) -> p h x", h=NH),
                                    func=AF.Copy, scale=1.0)
                                nc.vector.tensor_copy(out=dst[:, :, DEP:DEP + 1], in_=ones_sb)
                    # AV matmuls per head
                    av = {}
                    for h in heads:
                        av[h] = psAV.tile([65, S], F, tag="av", name=f"av{h}")
                        for kt in range(KT):
                            for cc in range(2):
                                sl = slice(cc * 512, (cc + 1) * 512)
                                nc.tensor.matmul(
                                    av[h][:, sl],
                                    vv[:, kt, h * 65:(h + 1) * 65],
                                    e_tiles[h][kt][:, sl],
                                    start=(kt == 0), stop=(kt == KT - 1))
                    rbhs = {}
                    for h in heads:
                        srow = rpool.tile([1, S], FR, tag="srow", name=f"sr{h}")
                        nc.scalar.activation(out=srow, in_=av[h][64:65, :], func=AF.Copy, scale=1.0)
                        # broadcast sums across partitions on the (idle) PE:
                        # ones_col.T @ srow  — K=1 outer product
                        sb_ps = psS.tile([128, S], F, tag="score", name=f"sbps{h}")
                        for cc in range(2):
                            sl = slice(cc * 512, (cc + 1) * 512)
                            nc.tensor.matmul(
                                sb_ps[:, sl], ones_fr, srow[:, sl], start=True, stop=True)
                        rb = rpool.tile([128, S], F, tag="rb", name=f"rb{h}")
                        nc.vector.reciprocal_approx_fast(out=rb, in_=sb_ps)
                        nc.vector.tensor_tensor(
                            out=oav[h][0:DEP, :], in0=av[h][0:DEP, :], in1=rb[0:DEP, :], op=OP.mult)
                        rbh = rpool.tile([128, S], F16, tag="rbh", name=f"rh{h}")
                        nc.vector.tensor_copy(out=rbh, in_=rb)
                        rbhs[h] = rbh

                    def mk(h, kt, rbh):
                        def go():
                            att_sb = apool.tile([128, S], F16, tag="att", name=f"at{h}_{kt}")
                            nc.vector.tensor_tensor(
                                out=att_sb, in0=e_tiles_all[h][kt], in1=rbh, op=OP.mult)
                            nc.sync.dma_start(
                                out=d["attT"][h, kt * 128:(kt + 1) * 128, :], in_=att_sb)
                        return go
                    tail_ops = [mk(h, kt, rbhs[h]) for kt in range(KT) for h in heads]
                    if hp < NH // 2 - 1:
                        pending.extend(tail_ops)
                    else:
                        for op in tail_ops:
                            op()
                for op in pending:
                    op()

        # ---------------- phase C: output projection ----------------
        with tc.tile_pool(name="psO", bufs=4, space="PSUM") as psO:
            for qt in range(NT):
                out_sb = osb_t[qt % 3]
                for nch in range(2):
                    po = psO.tile([128, 512], F, tag="po")
                    for h in range(NH):
                        nc.tensor.matmul(
                            po,
                            oav[h][:, qt * 128:(qt + 1) * 128],
                            wo_s[:, h, nch * 512:(nch + 1) * 512],
                            start=(h == 0), stop=(h == NH - 1))
                    nc.scalar.activation(
                        out=out_sb[:, nch * 512:(nch + 1) * 512], in_=po,
                        func=AF.Copy, scale=1.0)
                nc.scalar.dma_start(out=d["outp"][qt * 128:(qt + 1) * 128, :], in_=out_sb)


def _get_nc():
    if "nc" not in _CACHE:
        _CACHE["nc"] = _build()
    return _CACHE["nc"]


def kernel(v, k, q, mask, Wq0, bq0, Wk0, bk0, Wv, bv, Wo, bo):
    v = np.asarray(v, dtype=np.float32)
    k = np.asarray(k, dtype=np.float32)
    q = np.asarray(q, dtype=np.float32)
    mask = np.asarray(mask)
    Wq0 = np.asarray(Wq0, dtype=np.float32)
    Wk0 = np.asarray(Wk0, dtype=np.float32)
    Wv = np.asarray(Wv, dtype=np.float32)
    Wo = np.asarray(Wo, dtype=np.float32)
    bq0 = np.asarray(bq0, dtype=np.float32)
    bk0 = np.asarray(bk0, dtype=np.float32)
    bv = np.asarray(bv, dtype=np.float32)
    bo = np.asarray(bo, dtype=np.float32)
    B = v.shape[0]
    HTOT = 16

    nc = _get_nc()

    per_batch = []
    for b in range(B):
        qT = q[b, 1:, :].T.reshape(KT, 128, S)
        kT = k[b, :-1, :].T.reshape(KT, 128, S)
        vT = v[b].T.reshape(KT, 128, S).astype(np.float16)
        keepT = (1 - mask[b]).T.reshape(KT, 128, S).astype(np.float16)
        kvw = np.empty((KT, 128, 2304), dtype=np.float16)
        kvw[:, :, 0:S] = keepT
        kvw[:, :, S:2 * S] = vT
        per_batch.append((qT, kT, kvw))
    in_maps = []
    for c in range(8):
        b, g = c // 4, c % 4
        cols = slice(g * DCOL, (g + 1) * DCOL)
        qT, kT, kvw_b = per_batch[b]
        qkw = np.empty((KT, 128, 2560), dtype=np.float16)
        qkw[:, :, 0:DCOL] = Wq0[:, cols].reshape(KT, 128, DCOL)
        qkw[:, :, DCOL:2 * DCOL] = Wk0[:, cols].reshape(KT, 128, DCOL)
        qkw[:, :, 512:512 + S] = qT
        qkw[:, :, 512 + S:512 + 2 * S] = kT
        kvw = kvw_b.copy()
        kvw[:, :, 2 * S:2 * S + DCOL] = Wv[:, cols].reshape(KT, 128, DCOL).astype(np.float16)
        wo_dup = np.empty((128, NH, DIN), dtype=np.float32)
        wo_block = Wo[cols, :].reshape(NH, DEP, DIN).transpose(1, 0, 2)
        wo_dup[0:DEP] = wo_block
        wo_dup[DEP:128] = wo_block
        m = {
            "qkw": qkw, "kvw": kvw, "wo": wo_dup,
            "bq": np.ascontiguousarray(bq0[cols].reshape(NH, DEP).T),
            "bk": np.ascontiguousarray(bk0[cols].reshape(2, 128).T),
            "ident": _IDENT,
        }
        in_maps.append(m)

    res = run_bass_kernel_spmd(nc, in_maps, core_ids=list(range(8)))

    att = np.empty((B, HTOT, S, S), dtype=np.float64)
    out = np.empty((B, S, DIN), dtype=np.float64)
    bias_row = (bv.astype(np.float64) @ Wo.astype(np.float64)) + bo.astype(np.float64)
    for b in range(B):
        acc = None
        for g in range(4):
            r = res.results[b * 4 + g]
            attT = r["attT"]
            for hl in range(NH):
                att[b, g * NH + hl] = attT[hl].T
            acc = r["outp"].astype(np.float64) if acc is None else acc + r["outp"]
        out[b] = acc + bias_row[None, :]
    return out, att


# revision 39
# speedup vs baseline: 1.0360x; 1.0360x over previous
"""Trainium2 Bass kernel for nn_MultiHeadAttention_67757404062370.

Sharding: data-parallel over batch (2) x tensor-parallel over heads (4 groups
of 4 heads) = 8 NeuronCores. Core c handles batch c//4, heads 4*(c%4)..4*(c%4)+3.

Device-side per core (transposed layout throughout):
  kk^T = Wk_g^T k^T (+bk)  [128=2 heads x 64, 2, seq] fp32r
  qq^T = Wq_g^T q^T (+bq)  [128, 4, seq] fp32r, zero-padded in the other
         head's 64 rows so the scores contraction can use K=128 (K=64
         matmuls never warm the PE clock gate).
  vv   = v Wv_g  [seq, 4*65] fp16 with ones columns
  s^T[k,q] = kkT_tile.T @ qqT_pad   (K=128, fp32r)
  u = exp(s/8) fp16; t = max(u,1); e = t*keep  (exp(relu(x)) == max(exp(x),1))
  av^T (+ sums row via ones cols) = vv_aug.T @ e   (fp16, K=128)
  att^T = e * (1/sums) fp16 ; oav^T = av^T * (1/sums) fp32r
  outp[q,:] = sum_h oav_h^T.T @ Wo_rows_h  (K=64 per head, fp32r)
Host: gather, transpose att views, sum outp over the 4 head-group cores per
batch, add (bv @ Wo + bo), cast to float64.
"""

import sys

if "/opt/trn_rl_repo" not in sys.path:
    sys.path.insert(0, "/opt/trn_rl_repo")

import numpy as np

import concourse.bacc as bacc
import concourse.tile as tile
from concourse import mybir
from concourse.bass_utils import run_bass_kernel_spmd

F = mybir.dt.float32
FR = mybir.dt.float32r
F16 = mybir.dt.float16
AF = mybir.ActivationFunctionType
OP = mybir.AluOpType

S = 1024
DIN = 1024
NH = 4
DEP = 64
DCOL = NH * DEP
NT = S // 128
KT = DIN // 128

_CACHE = {}
_DEBUG = False
_IDENT = np.eye(128, dtype=np.float16)


def _build():
    nc = bacc.Bacc("TRN2", target_bir_lowering=False, debug=False, num_devices=8)

    d = {}
    # packed per-ktile input streams: one large DMA per k-tile
    d["qkw"] = nc.dram_tensor("qkw", [KT, 128, 2560], F16, kind="ExternalInput").ap()
    d["kvw"] = nc.dram_tensor("kvw", [KT, 128, 2304], F16, kind="ExternalInput").ap()
    d["wo"] = nc.dram_tensor("wo", [128, NH, DIN], FR, kind="ExternalInput").ap()
    d["bq"] = nc.dram_tensor("bq", [DEP, NH], F, kind="ExternalInput").ap()
    d["bk"] = nc.dram_tensor("bk", [128, 2], F, kind="ExternalInput").ap()
    d["ident"] = nc.dram_tensor("ident", [128, 128], F16, kind="ExternalInput").ap()
    d["attT"] = nc.dram_tensor("attT", [NH, S, S], F16, kind="ExternalOutput").ap()
    d["outp"] = nc.dram_tensor("outp", [S, DIN], F, kind="ExternalOutput").ap()
    if _DEBUG:
        d["dbg_av"] = nc.dram_tensor("dbg_av", [NH, 65, S], F, kind="ExternalOutput").ap()
        d["dbg_oav"] = nc.dram_tensor("dbg_oav", [NH, DEP, S], F, kind="ExternalOutput").ap()

    with tile.TileContext(nc) as tc:
        _emit(nc, tc, d)
    nc.compile()
    return nc


def _emit(nc, tc, d):
    from contextlib import ExitStack

    ctx = ExitStack()
    with ctx:
        # ---------------- persistent tiles ----------------
        persist = ctx.enter_context(tc.tile_pool(name="persist", bufs=1))
        # q proj, zero-padded per head to a full 128-row contraction
        qqT = persist.tile([128, NH, S], FR, tag="qqT")
        # k proj, two heads stacked per dcol-tile
        kkT = persist.tile([128, 2, S], FR, tag="kkT")
        vv = persist.tile([128, NT, NH * 65], F16, tag="vv")
        wo_s = persist.tile([128, NH, DIN], FR, tag="wo")
        kvw_s = persist.tile([128, KT, 2304], F16, tag="kvw")
        keep_s = [kvw_s[:, i, 0:S] for i in range(KT)]
        vt_s = [kvw_s[:, i, S:2 * S] for i in range(KT)]
        wv_sl = [kvw_s[:, i, 2 * S:2 * S + DCOL] for i in range(KT)]
        mneg_s = [persist.tile([128, S], F16, tag=f"mneg{i}", name=f"mneg{i}") for i in range(KT)]
        ident_s = persist.tile([128, 128], F16, tag="ident")
        bq_s = persist.tile([DEP, NH], F, tag="bq")
        bk_s = persist.tile([128, 2], F, tag="bk")
        ones_sb = persist.tile([128, NH, 1], F, tag="ones")
        nc.vector.memset(ones_sb, 1.0)
        zscr = persist.tile([128, S], F, tag="zscr")
        nc.vector.memset(zscr, 0.0)
        m60k = persist.tile([128, 1], F, tag="m60k")
        nc.vector.memset(m60k, -60000.0)
        ones_row = persist.tile([1, 128], F, tag="ones_row")
        nc.vector.memset(ones_row, 1.0)
        ones_fr = persist.tile([1, 128], FR, tag="ones_fr")
        nc.vector.tensor_copy(out=ones_fr, in_=ones_row)
        oav = [persist.tile([128, S], FR, tag=f"oav{h}", name=f"oav{h}") for h in range(NH)]
        osb_t = [persist.tile([128, DIN], F, tag=f"osb{i}", name=f"osb{i}") for i in range(3)]

        nc.sync.dma_start(out=bq_s, in_=d["bq"])
        nc.sync.dma_start(out=ident_s, in_=d["ident"])
        # zero fills from the scratch, before any data arrives
        for h in range(NH):
            zw = slice((1 - (h % 2)) * DEP, (2 - (h % 2)) * DEP)
            nc.vector.tensor_copy(out=qqT[zw, h, :], in_=zscr[zw, :])
            nc.vector.tensor_copy(out=oav[h][DEP:128, :], in_=zscr[DEP:128, :])
        nc.sync.dma_start(out=bk_s, in_=d["bk"])

        # ---------------- phase A: q/k projections ----------------
        with tc.tile_pool(name="projin", bufs=4) as pin:
            with tc.tile_pool(name="psA1", bufs=4, space="PSUM") as psA1:
                ps_q = [psA1.tile([128, S], F, tag="psA1", name=f"psq{i}") for i in range(2)]
                ps_k = [psA1.tile([128, S], F, tag="psA1", name=f"psk{i}") for i in range(2)]
                for kt in range(KT):
                    qkw = pin.tile([128, 2560], F16, tag="pin", name=f"qkw{kt}")
                    nc.sync.dma_start(out=qkw, in_=d["qkw"][kt])
                    for dc in range(2):
                        for cc in range(2):
                            sl = slice(512 + cc * 512, 512 + (cc + 1) * 512)
                            nc.tensor.matmul(
                                ps_q[dc][:, slice(cc * 512, (cc + 1) * 512)],
                                qkw[:, dc * 128:(dc + 1) * 128],
                                qkw[:, sl],
                                start=(kt == 0), stop=(kt == KT - 1))
                            nc.tensor.matmul(
                                ps_k[dc][:, slice(cc * 512, (cc + 1) * 512)],
                                qkw[:, 256 + dc * 128:256 + (dc + 1) * 128],
                                qkw[:, 1024 + sl.start:1024 + sl.stop],
                                start=(kt == 0), stop=(kt == KT - 1))
                # mask/v/wv loads queue behind the q/k stream
                for kt in range(KT):
                    nc.sync.dma_start(out=kvw_s[:, kt, :], in_=d["kvw"][kt])
                # evacuate: kk on DVE, qq on ACT — both unblock scores fast
                for dc in range(2):
                    nc.vector.tensor_scalar_add(
                        out=kkT[:, dc, :], in0=ps_k[dc], scalar1=bk_s[:, dc:dc + 1])
                    for hf in range(2):
                        h = dc * 2 + hf
                        rw = slice(hf * DEP, (hf + 1) * DEP)
                        nc.scalar.activation(
                            out=qqT[rw, h, :], in_=ps_q[dc][rw, :],
                            func=AF.Identity, bias=bq_s[:, h:h + 1], scale=1.0)


        # wo: host-packed with duplicated head rows (see phase C)
        if True:
            nc.sync.dma_start(out=wo_s, in_=d["wo"])

            # ---------------- phase B: heads in pairs ----------------
            with tc.tile_pool(name="escore", bufs=16) as epool, \
                 tc.tile_pool(name="utile", bufs=3) as upool, \
                 tc.tile_pool(name="attsb", bufs=5) as apool, \
                 tc.tile_pool(name="rtiles", bufs=2) as rpool, \
                 tc.tile_pool(name="psS", bufs=2, space="PSUM") as psS, \
                 tc.tile_pool(name="psAV", bufs=2, space="PSUM") as psAV:
                pending = []  # deferred DVE tail work from the previous pair
                e_tiles_all = {}
                for hp in range(NH // 2):
                    heads = (2 * hp, 2 * hp + 1)
                    e_tiles = {h: [] for h in heads}
                    # scores + exp + mask for both heads of the pair
                    for kt in range(KT):
                        if hp == 0:
                            # maskneg = (keep - 1) * 60000, derived just-in-time
                            nc.vector.tensor_scalar(
                                out=mneg_s[kt], in0=keep_s[kt], scalar1=-1.0,
                                scalar2=60000.0, op0=OP.add, op1=OP.mult)
                        pstiles = {}
                        for h in heads:
                            if pending:
                                pending.pop(0)()
                            ps = psS.tile([128, S], F, tag="score", name=f"sc{h}_{kt}")
                            pstiles[h] = ps
                            for cc in range(2):
                                sl = slice(cc * 512, (cc + 1) * 512)
                                nc.tensor.matmul(
                                    ps[:, sl],
                                    kkT[:, hp, kt * 128:(kt + 1) * 128],
                                    qqT[:, h, sl],
                                    start=True, stop=False)
                        for h in heads:
                            ps = pstiles[h]
                            for cc in range(2):
                                sl = slice(cc * 512, (cc + 1) * 512)
                                nc.tensor.matmul(
                                    ps[:, sl],
                                    ident_s,
                                    mneg_s[kt][:, sl],
                                    start=False, stop=True)
                        for h in heads:
                            ps = pstiles[h]
                            u = upool.tile([128, S], F16, tag="u", name=f"u{h}_{kt}")
                            nc.scalar.activation(out=u, in_=ps, func=AF.Exp, scale=0.125)
                            # masked u is exactly 0, so max(u, keep) applies both
                            # the exp-domain relu (max with 1) and the mask
                            e = epool.tile([128, S], F16, tag="e", name=f"e{h}_{kt}")
                            nc.vector.tensor_tensor(
                                out=e, in0=u, in1=keep_s[kt], op=OP.max)
                            e_tiles[h].append(e)
                            e_tiles_all[h] = e_tiles[h]
                        # vv projection interleaved into the back half of S0:
                        # fills the PE's exp-wait gaps; av-pool slots rotate
                        if hp == 0 and kt >= KT // 2:
                            for st in (2 * (kt - KT // 2), 2 * (kt - KT // 2) + 1):
                                pv = psAV.tile([128, 256], F, tag="av", name=f"psv{st}")
                                for vkt in range(KT):
                                    nc.tensor.matmul(
                                        pv,
                                        vt_s[vkt][:, st * 128:(st + 1) * 128],
                                        wv_sl[vkt],
                                        start=(vkt == 0), stop=(vkt == KT - 1))
                                dst = vv[:, st, :].rearrange("p (h x) -> p h x", h=NH)
                                nc.scalar.activation(
                                    out=dst[:, :, 0:DEP],
                                    in_=pv.rearrange("p (h x) -> p h x", h=NH),
                                    func=AF.Copy, scale=1.0)
                                nc.vector.tensor_copy(out=dst[:, :, DEP:DEP + 1], in_=ones_sb)
                    # AV matmuls per head
                    av = {}
                    for h in heads:
                        av[h] = psAV.tile([65, S], F, tag="av", name=f"av{h}")
                        for kt in range(KT):
                            for cc in range(2):
                                sl = slice(cc * 512, (cc + 1) * 512)
                                nc.tensor.matmul(
                                    av[h][:, sl],
                                    vv[:, kt, h * 65:(h + 1) * 65],
                                    e_tiles[h][kt][:, sl],
                                    start=(kt == 0), stop=(kt == KT - 1))
                    rbhs = {}
                    for h in heads:
                        srow = rpool.tile([1, S], FR, tag="srow", name=f"sr{h}")
                        nc.scalar.activation(out=srow, in_=av[h][64:65, :], func=AF.Copy, scale=1.0)
                        # broadcast sums across partitions on the (idle) PE:
                        # ones_col.T @ srow  — K=1 outer product
                        sb_ps = psS.tile([128, S], F, tag="score", name=f"sbps{h}")
                        for cc in range(2):
                            sl = slice(cc * 512, (cc + 1) * 512)
                            nc.tensor.matmul(
                                sb_ps[:, sl], ones_fr, srow[:, sl], start=True, stop=True)
                        rb = rpool.tile([128, S], F, tag="rb", name=f"rb{h}")
                        nc.vector.reciprocal_approx_fast(out=rb, in_=sb_ps)
                        nc.vector.tensor_tensor(
                            out=oav[h][0:DEP, :], in0=av[h][0:DEP, :], in1=rb[0:DEP, :], op=OP.mult)
                        rbh = rpool.tile([128, S], F16, tag="rbh", name=f"rh{h}")
                        nc.vector.tensor_copy(out=rbh, in_=rb)
                        rbhs[h] = rbh

                    def mk(h, kt, rbh):
                        def go():
                            att_sb = apool.tile([128, S], F16, tag="att", name=f"at{h}_{kt}")
                            nc.vector.tensor_tensor(
                                out=att_sb, in0=e_tiles_all[h][kt], in1=rbh, op=OP.mult)
                            nc.sync.dma_start(
                                out=d["attT"][h, kt * 128:(kt + 1) * 128, :], in_=att_sb)
                        return go
                    tail_ops = [mk(h, kt, rbhs[h]) for kt in range(KT) for h in heads]
                    if hp < NH // 2 - 1:
                        pending.extend(tail_ops)
                    else:
                        for op in tail_ops:
                            op()
                for op in pending:
                    op()

        # ---------------- phase C: output projection ----------------
        with tc.tile_pool(name="psO", bufs=4, space="PSUM") as psO:
            for qt in range(NT):
                out_sb = osb_t[qt % 3]
                for nch in range(2):
                    po = psO.tile([128, 512], F, tag="po")
                    for h in range(NH):
                        nc.tensor.matmul(
                            po,
                            oav[h][:, qt * 128:(qt + 1) * 128],
                            wo_s[:, h, nch * 512:(nch + 1) * 512],
                            start=(h == 0), stop=(h == NH - 1))
                    nc.scalar.activation(
                        out=out_sb[:, nch * 512:(nch + 1) * 512], in_=po,
                        func=AF.Copy, scale=1.0)
                nc.scalar.dma_start(out=d["outp"][qt * 128:(qt + 1) * 128, :], in_=out_sb)


def _get_nc():
    if "nc" not in _CACHE:
        _CACHE["nc"] = _build()
    return _CACHE["nc"]


def kernel(v, k, q, mask, Wq0, bq0, Wk0, bk0, Wv, bv, Wo, bo):
    v = np.asarray(v, dtype=np.float32)
    k = np.asarray(k, dtype=np.float32)
    q = np.asarray(q, dtype=np.float32)
    mask = np.asarray(mask)
    Wq0 = np.asarray(Wq0, dtype=np.float32)
    Wk0 = np.asarray(Wk0, dtype=np.float32)
    Wv = np.asarray(Wv, dtype=np.float32)
    Wo = np.asarray(Wo, dtype=np.float32)
    bq0 = np.asarray(bq0, dtype=np.float32)
    bk0 = np.asarray(bk0, dtype=np.float32)
    bv = np.asarray(bv, dtype=np.float32)
    bo = np.asarray(bo, dtype=np.float32)
    B = v.shape[0]
    HTOT = 16

    nc = _get_nc()

    per_batch = []
    for b in range(B):
        qT = q[b, 1:, :].T.reshape(KT, 128, S)
        kT = k[b, :-1, :].T.reshape(KT, 128, S)
        vT = v[b].T.reshape(KT, 128, S).astype(np.float16)
        keepT = (1 - mask[b]).T.reshape(KT, 128, S).astype(np.float16)
        kvw = np.empty((KT, 128, 2304), dtype=np.float16)
        kvw[:, :, 0:S] = keepT
        kvw[:, :, S:2 * S] = vT
        per_batch.append((qT, kT, kvw))
    in_maps = []
    for c in range(8):
        b, g = c // 4, c % 4
        cols = slice(g * DCOL, (g + 1) * DCOL)
        qT, kT, kvw_b = per_batch[b]
        qkw = np.empty((KT, 128, 2560), dtype=np.float16)
        qkw[:, :, 0:DCOL] = Wq0[:, cols].reshape(KT, 128, DCOL)
        qkw[:, :, DCOL:2 * DCOL] = Wk0[:, cols].reshape(KT, 128, DCOL)
        qkw[:, :, 512:512 + S] = qT
        qkw[:, :, 512 + S:512 + 2 * S] = kT
        kvw = kvw_b.copy()
        kvw[:, :, 2 * S:2 * S + DCOL] = Wv[:, cols].reshape(KT, 128, DCOL).astype(np.float16)
        wo_dup = np.empty((128, NH, DIN), dtype=np.float32)
        wo_block = Wo[cols, :].reshape(NH, DEP, DIN).transpose(1, 0, 2)
        wo_dup[0:DEP] = wo_block
        wo_dup[DEP:128] = wo_block
        m = {
            "qkw": qkw, "kvw": kvw, "wo": wo_dup,
            "bq": np.ascontiguousarray(bq0[cols].reshape(NH, DEP).T),
            "bk": np.ascontiguousarray(bk0[cols].reshape(2, 128).T),
            "ident": _IDENT,
        }
        in_maps.append(m)

    res = run_bass_kernel_spmd(nc, in_maps, core_ids=list(range(8)))

    att = np.empty((B, HTOT, S, S), dtype=np.float64)
    out = np.empty((B, S, DIN), dtype=np.float64)
    bias_row = (bv.astype(np.float64) @ Wo.astype(np.float64)) + bo.astype(np.float64)
    for b in range(B):
        acc = None
        for g in range(4):
            r = res.results[b * 4 + g]
            attT = r["attT"]
            for hl in range(NH):
                att[b, g * NH + hl] = attT[hl].T
            acc = r["outp"].astype(np.float64) if acc is None else acc + r["outp"]
        out[b] = acc + bias_row[None, :]
    return out, att


# revision 40
# speedup vs baseline: 1.0493x; 1.0128x over previous
"""Trainium2 Bass kernel for nn_MultiHeadAttention_67757404062370.

Sharding: data-parallel over batch (2) x tensor-parallel over heads (4 groups
of 4 heads) = 8 NeuronCores. Core c handles batch c//4, heads 4*(c%4)..4*(c%4)+3.

Device-side per core (transposed layout throughout):
  kk^T = Wk_g^T k^T (+bk)  [128=2 heads x 64, 2, seq] fp32r
  qq^T = Wq_g^T q^T (+bq)  [128, 4, seq] fp32r, zero-padded in the other
         head's 64 rows so the scores contraction can use K=128 (K=64
         matmuls never warm the PE clock gate).
  vv   = v Wv_g  [seq, 4*65] fp16 with ones columns
  s^T[k,q] = kkT_tile.T @ qqT_pad   (K=128, fp32r)
  u = exp(s/8) fp16; t = max(u,1); e = t*keep  (exp(relu(x)) == max(exp(x),1))
  av^T (+ sums row via ones cols) = vv_aug.T @ e   (fp16, K=128)
  att^T = e * (1/sums) fp16 ; oav^T = av^T * (1/sums) fp32r
  outp[q,:] = sum_h oav_h^T.T @ Wo_rows_h  (K=64 per head, fp32r)
Host: gather, transpose att views, sum outp over the 4 head-group cores per
batch, add (bv @ Wo + bo), cast to float64.
"""

import sys

if "/opt/trn_rl_repo" not in sys.path:
    sys.path.insert(0, "/opt/trn_rl_repo")

import numpy as np

import concourse.bacc as bacc
import concourse.tile as tile
from concourse import mybir
from concourse.bass_utils import run_bass_kernel_spmd

F = mybir.dt.float32
FR = mybir.dt.float32r
F16 = mybir.dt.float16
AF = mybir.ActivationFunctionType
OP = mybir.AluOpType

S = 1024
DIN = 1024
NH = 4
DEP = 64
DCOL = NH * DEP
NT = S // 128
KT = DIN // 128

_CACHE = {}
_DEBUG = False
_IDENT = np.eye(128, dtype=np.float16)


def _build():
    nc = bacc.Bacc("TRN2", target_bir_lowering=False, debug=False, num_devices=8)

    d = {}
    # packed per-ktile input streams: one large DMA per k-tile
    d["qkw"] = nc.dram_tensor("qkw", [KT, 128, 2560], F16, kind="ExternalInput").ap()
    d["kvw"] = nc.dram_tensor("kvw", [KT, 128, 2304], F16, kind="ExternalInput").ap()
    d["wo"] = nc.dram_tensor("wo", [128, 2, DIN], FR, kind="ExternalInput").ap()
    d["bq"] = nc.dram_tensor("bq", [DEP, NH], F, kind="ExternalInput").ap()
    d["bk"] = nc.dram_tensor("bk", [128, 2], F, kind="ExternalInput").ap()
    d["ident"] = nc.dram_tensor("ident", [128, 128], F16, kind="ExternalInput").ap()
    d["attT"] = nc.dram_tensor("attT", [NH, S, S], F16, kind="ExternalOutput").ap()
    d["outp"] = nc.dram_tensor("outp", [S, DIN], F16, kind="ExternalOutput").ap()
    if _DEBUG:
        d["dbg_av"] = nc.dram_tensor("dbg_av", [NH, 65, S], F, kind="ExternalOutput").ap()
        d["dbg_oav"] = nc.dram_tensor("dbg_oav", [NH, DEP, S], F, kind="ExternalOutput").ap()

    with tile.TileContext(nc) as tc:
        _emit(nc, tc, d)
    nc.compile()
    return nc


def _emit(nc, tc, d):
    from contextlib import ExitStack

    ctx = ExitStack()
    with ctx:
        # ---------------- persistent tiles ----------------
        persist = ctx.enter_context(tc.tile_pool(name="persist", bufs=1))
        # q proj, zero-padded per head to a full 128-row contraction
        qqT = persist.tile([128, NH, S], FR, tag="qqT")
        # k proj, two heads stacked per dcol-tile
        kkT = persist.tile([128, 2, S], FR, tag="kkT")
        vv = persist.tile([128, NT, NH * 65], F16, tag="vv")
        wo_s = persist.tile([128, 2, DIN], FR, tag="wo")
        kvw_s = persist.tile([128, KT, 2304], F16, tag="kvw")
        keep_s = [kvw_s[:, i, 0:S] for i in range(KT)]
        vt_s = [kvw_s[:, i, S:2 * S] for i in range(KT)]
        wv_sl = [kvw_s[:, i, 2 * S:2 * S + DCOL] for i in range(KT)]
        mneg_s = [persist.tile([128, S], F16, tag=f"mneg{i}", name=f"mneg{i}") for i in range(KT)]
        ident_s = persist.tile([128, 128], F16, tag="ident")
        bq_s = persist.tile([DEP, NH], F, tag="bq")
        bk_s = persist.tile([128, 2], F, tag="bk")
        ones_sb = persist.tile([128, NH, 1], F, tag="ones")
        nc.vector.memset(ones_sb, 1.0)
        zscr = persist.tile([128, S], F, tag="zscr")
        nc.vector.memset(zscr, 0.0)
        m60k = persist.tile([128, 1], F, tag="m60k")
        nc.vector.memset(m60k, -60000.0)
        ones_row = persist.tile([1, 128], F, tag="ones_row")
        nc.vector.memset(ones_row, 1.0)
        ones_fr = persist.tile([1, 128], FR, tag="ones_fr")
        nc.vector.tensor_copy(out=ones_fr, in_=ones_row)
        oav = [persist.tile([128, S], FR, tag=f"oav{h}", name=f"oav{h}") for h in range(2)]
        osb_t = [persist.tile([128, DIN], F16, tag=f"osb{i}", name=f"osb{i}") for i in range(3)]

        nc.sync.dma_start(out=bq_s, in_=d["bq"])
        nc.sync.dma_start(out=ident_s, in_=d["ident"])
        # zero fills from the scratch, before any data arrives
        for h in range(NH):
            zw = slice((1 - (h % 2)) * DEP, (2 - (h % 2)) * DEP)
            nc.vector.tensor_copy(out=qqT[zw, h, :], in_=zscr[zw, :])
        nc.sync.dma_start(out=bk_s, in_=d["bk"])

        # ---------------- phase A: q/k projections ----------------
        with tc.tile_pool(name="projin", bufs=4) as pin:
            with tc.tile_pool(name="psA1", bufs=4, space="PSUM") as psA1:
                ps_q = [psA1.tile([128, S], F, tag="psA1", name=f"psq{i}") for i in range(2)]
                ps_k = [psA1.tile([128, S], F, tag="psA1", name=f"psk{i}") for i in range(2)]
                for kt in range(KT):
                    qkw = pin.tile([128, 2560], F16, tag="pin", name=f"qkw{kt}")
                    nc.sync.dma_start(out=qkw, in_=d["qkw"][kt])
                    for dc in range(2):
                        for cc in range(2):
                            sl = slice(512 + cc * 512, 512 + (cc + 1) * 512)
                            nc.tensor.matmul(
                                ps_q[dc][:, slice(cc * 512, (cc + 1) * 512)],
                                qkw[:, dc * 128:(dc + 1) * 128],
                                qkw[:, sl],
                                start=(kt == 0), stop=(kt == KT - 1))
                            nc.tensor.matmul(
                                ps_k[dc][:, slice(cc * 512, (cc + 1) * 512)],
                                qkw[:, 256 + dc * 128:256 + (dc + 1) * 128],
                                qkw[:, 1024 + sl.start:1024 + sl.stop],
                                start=(kt == 0), stop=(kt == KT - 1))
                # mask/v/wv loads queue behind the q/k stream
                for kt in range(KT):
                    nc.sync.dma_start(out=kvw_s[:, kt, :], in_=d["kvw"][kt])
                # evacuate: kk on DVE, qq on ACT — both unblock scores fast
                for dc in range(2):
                    nc.vector.tensor_scalar_add(
                        out=kkT[:, dc, :], in0=ps_k[dc], scalar1=bk_s[:, dc:dc + 1])
                    for hf in range(2):
                        h = dc * 2 + hf
                        rw = slice(hf * DEP, (hf + 1) * DEP)
                        nc.scalar.activation(
                            out=qqT[rw, h, :], in_=ps_q[dc][rw, :],
                            func=AF.Identity, bias=bq_s[:, h:h + 1], scale=1.0)


        # wo: host-packed with duplicated head rows (see phase C)
        if True:
            nc.sync.dma_start(out=wo_s, in_=d["wo"])

            # ---------------- phase B: heads in pairs ----------------
            with tc.tile_pool(name="escore", bufs=16) as epool, \
                 tc.tile_pool(name="utile", bufs=3) as upool, \
                 tc.tile_pool(name="attsb", bufs=5) as apool, \
                 tc.tile_pool(name="rtiles", bufs=2) as rpool, \
                 tc.tile_pool(name="psS", bufs=2, space="PSUM") as psS, \
                 tc.tile_pool(name="psAV", bufs=2, space="PSUM") as psAV:
                pending = []  # deferred DVE tail work from the previous pair
                e_tiles_all = {}
                for hp in range(NH // 2):
                    heads = (2 * hp, 2 * hp + 1)
                    e_tiles = {h: [] for h in heads}
                    # scores + exp + mask for both heads of the pair
                    for kt in range(KT):
                        if hp == 0:
                            # maskneg = (keep - 1) * 60000, derived just-in-time
                            nc.vector.tensor_scalar(
                                out=mneg_s[kt], in0=keep_s[kt], scalar1=-1.0,
                                scalar2=60000.0, op0=OP.add, op1=OP.mult)
                        pstiles = {}
                        for h in heads:
                            if pending:
                                pending.pop(0)()
                            ps = psS.tile([128, S], F, tag="score", name=f"sc{h}_{kt}")
                            pstiles[h] = ps
                            for cc in range(2):
                                sl = slice(cc * 512, (cc + 1) * 512)
                                nc.tensor.matmul(
                                    ps[:, sl],
                                    kkT[:, hp, kt * 128:(kt + 1) * 128],
                                    qqT[:, h, sl],
                                    start=True, stop=False)
                        for h in heads:
                            ps = pstiles[h]
                            for cc in range(2):
                                sl = slice(cc * 512, (cc + 1) * 512)
                                nc.tensor.matmul(
                                    ps[:, sl],
                                    ident_s,
                                    mneg_s[kt][:, sl],
                                    start=False, stop=True)
                        for h in heads:
                            ps = pstiles[h]
                            u = upool.tile([128, S], F16, tag="u", name=f"u{h}_{kt}")
                            nc.scalar.activation(out=u, in_=ps, func=AF.Exp, scale=0.125)
                            # masked u is exactly 0, so max(u, keep) applies both
                            # the exp-domain relu (max with 1) and the mask
                            e = epool.tile([128, S], F16, tag="e", name=f"e{h}_{kt}")
                            nc.vector.tensor_tensor(
                                out=e, in0=u, in1=keep_s[kt], op=OP.max)
                            e_tiles[h].append(e)
                            e_tiles_all[h] = e_tiles[h]
                        # vv projection interleaved into the back half of S0:
                        # fills the PE's exp-wait gaps; av-pool slots rotate
                        if hp == 0 and kt >= KT // 2:
                            for st in (2 * (kt - KT // 2), 2 * (kt - KT // 2) + 1):
                                pv = psAV.tile([128, 256], F, tag="av", name=f"psv{st}")
                                for vkt in range(KT):
                                    nc.tensor.matmul(
                                        pv,
                                        vt_s[vkt][:, st * 128:(st + 1) * 128],
                                        wv_sl[vkt],
                                        start=(vkt == 0), stop=(vkt == KT - 1))
                                dst = vv[:, st, :].rearrange("p (h x) -> p h x", h=NH)
                                nc.scalar.activation(
                                    out=dst[:, :, 0:DEP],
                                    in_=pv.rearrange("p (h x) -> p h x", h=NH),
                                    func=AF.Copy, scale=1.0)
                                nc.vector.tensor_copy(out=dst[:, :, DEP:DEP + 1], in_=ones_sb)
                    # AV matmuls per head
                    av = {}
                    for h in heads:
                        av[h] = psAV.tile([65, S], F, tag="av", name=f"av{h}")
                        for kt in range(KT):
                            for cc in range(2):
                                sl = slice(cc * 512, (cc + 1) * 512)
                                nc.tensor.matmul(
                                    av[h][:, sl],
                                    vv[:, kt, h * 65:(h + 1) * 65],
                                    e_tiles[h][kt][:, sl],
                                    start=(kt == 0), stop=(kt == KT - 1))
                    rbhs = {}
                    for h in heads:
                        srow = rpool.tile([1, S], FR, tag="srow", name=f"sr{h}")
                        nc.scalar.activation(out=srow, in_=av[h][64:65, :], func=AF.Copy, scale=1.0)
                        # broadcast sums across partitions on the (idle) PE:
                        # ones_col.T @ srow  — K=1 outer product
                        sb_ps = psS.tile([128, S], F, tag="score", name=f"sbps{h}")
                        for cc in range(2):
                            sl = slice(cc * 512, (cc + 1) * 512)
                            nc.tensor.matmul(
                                sb_ps[:, sl], ones_fr, srow[:, sl], start=True, stop=True)
                        rb = rpool.tile([128, S], F, tag="rb", name=f"rb{h}")
                        nc.vector.reciprocal_approx_fast(out=rb, in_=sb_ps)
                        orows = slice(0, DEP) if h % 2 == 0 else slice(DEP, 128)
                        nc.vector.tensor_tensor(
                            out=oav[h // 2][orows, :], in0=av[h][0:DEP, :],
                            in1=rb[0:DEP, :], op=OP.mult)
                        rbh = rpool.tile([128, S], F16, tag="rbh", name=f"rh{h}")
                        nc.vector.tensor_copy(out=rbh, in_=rb)
                        rbhs[h] = rbh

                    def mk(h, kt, rbh):
                        def go():
                            att_sb = apool.tile([128, S], F16, tag="att", name=f"at{h}_{kt}")
                            nc.vector.tensor_tensor(
                                out=att_sb, in0=e_tiles_all[h][kt], in1=rbh, op=OP.mult)
                            nc.sync.dma_start(
                                out=d["attT"][h, kt * 128:(kt + 1) * 128, :], in_=att_sb)
                        return go
                    tail_ops = [mk(h, kt, rbhs[h]) for kt in range(KT) for h in heads]
                    if hp < NH // 2 - 1:
                        pending.extend(tail_ops)
                    else:
                        for op in tail_ops:
                            op()
                for op in pending:
                    op()

        # ---------------- phase C: output projection ----------------
        with tc.tile_pool(name="psO", bufs=4, space="PSUM") as psO:
            for qt in range(NT):
                out_sb = osb_t[qt % 3]
                for nch in range(2):
                    po = psO.tile([128, 512], F, tag="po")
                    for hp in range(2):
                        nc.tensor.matmul(
                            po,
                            oav[hp][:, qt * 128:(qt + 1) * 128],
                            wo_s[:, hp, nch * 512:(nch + 1) * 512],
                            start=(hp == 0), stop=(hp == 1))
                    nc.scalar.activation(
                        out=out_sb[:, nch * 512:(nch + 1) * 512], in_=po,
                        func=AF.Copy, scale=1.0)
                nc.scalar.dma_start(out=d["outp"][qt * 128:(qt + 1) * 128, :], in_=out_sb)


def _get_nc():
    if "nc" not in _CACHE:
        _CACHE["nc"] = _build()
    return _CACHE["nc"]


def kernel(v, k, q, mask, Wq0, bq0, Wk0, bk0, Wv, bv, Wo, bo):
    v = np.asarray(v, dtype=np.float32)
    k = np.asarray(k, dtype=np.float32)
    q = np.asarray(q, dtype=np.float32)
    mask = np.asarray(mask)
    Wq0 = np.asarray(Wq0, dtype=np.float32)
    Wk0 = np.asarray(Wk0, dtype=np.float32)
    Wv = np.asarray(Wv, dtype=np.float32)
    Wo = np.asarray(Wo, dtype=np.float32)
    bq0 = np.asarray(bq0, dtype=np.float32)
    bk0 = np.asarray(bk0, dtype=np.float32)
    bv = np.asarray(bv, dtype=np.float32)
    bo = np.asarray(bo, dtype=np.float32)
    B = v.shape[0]
    HTOT = 16

    nc = _get_nc()

    per_batch = []
    for b in range(B):
        qT = q[b, 1:, :].T.reshape(KT, 128, S)
        kT = k[b, :-1, :].T.reshape(KT, 128, S)
        vT = v[b].T.reshape(KT, 128, S).astype(np.float16)
        keepT = (1 - mask[b]).T.reshape(KT, 128, S).astype(np.float16)
        kvw = np.empty((KT, 128, 2304), dtype=np.float16)
        kvw[:, :, 0:S] = keepT
        kvw[:, :, S:2 * S] = vT
        per_batch.append((qT, kT, kvw))
    in_maps = []
    for c in range(8):
        b, g = c // 4, c % 4
        cols = slice(g * DCOL, (g + 1) * DCOL)
        qT, kT, kvw_b = per_batch[b]
        qkw = np.empty((KT, 128, 2560), dtype=np.float16)
        qkw[:, :, 0:DCOL] = Wq0[:, cols].reshape(KT, 128, DCOL)
        qkw[:, :, DCOL:2 * DCOL] = Wk0[:, cols].reshape(KT, 128, DCOL)
        qkw[:, :, 512:512 + S] = qT
        qkw[:, :, 512 + S:512 + 2 * S] = kT
        kvw = kvw_b.copy()
        kvw[:, :, 2 * S:2 * S + DCOL] = Wv[:, cols].reshape(KT, 128, DCOL).astype(np.float16)
        wo_pair = np.ascontiguousarray(
            Wo[cols, :].reshape(2, 128, DIN).transpose(1, 0, 2))
        m = {
            "qkw": qkw, "kvw": kvw, "wo": wo_pair,
            "bq": np.ascontiguousarray(bq0[cols].reshape(NH, DEP).T),
            "bk": np.ascontiguousarray(bk0[cols].reshape(2, 128).T),
            "ident": _IDENT,
        }
        in_maps.append(m)

    res = run_bass_kernel_spmd(nc, in_maps, core_ids=list(range(8)))

    att = np.empty((B, HTOT, S, S), dtype=np.float64)
    out = np.empty((B, S, DIN), dtype=np.float64)
    bias_row = (bv.astype(np.float64) @ Wo.astype(np.float64)) + bo.astype(np.float64)
    for b in range(B):
        acc = None
        for g in range(4):
            r = res.results[b * 4 + g]
            attT = r["attT"]
            for hl in range(NH):
                att[b, g * NH + hl] = attT[hl].T
            acc = r["outp"].astype(np.float64) if acc is None else acc + r["outp"]
        out[b] = acc + bias_row[None, :]
    return out, att
